# revision 47
# baseline (speedup 1.0000x reference)
"""Trainium2 Bass kernel for nn_BilinearScorer.

Reference computation (per full input):
    t = text @ W_text.T + b_text            # [B, H]
    v = t @ W_patch                         # [B, PD]
    scores[b, n] = patches[b, n, :] . v[b]  + t[b] . b_patch   # [B, N]

Strategy: data-parallel over batch B across 8 NeuronCores (4 batches/core).
The heavy op (patches . v) is HBM-bandwidth bound (64 MiB patches per core).
Per core:
  - all loads ride the gpsimd SWDGE ring in priority order (W_text chunk 0,
    b_text, replicated text rows, remaining W_text, W_patch, b_patch, then
    the 32 patch tiles): the ring is FIFO, so preamble tensors stream at
    full rate before the patch flood instead of being starved behind it.
    The ACT ring only writes scores back; ACT itself only does PSUM->SBUF
    copies, so no DMA issue ever blocks on a cross-engine wait.
  - patches are cast fp32->bf16 inline by the SWDGE DMA datapath, and rows
    are remapped so each partition reads one contiguous 16 KiB span per
    2 MiB tile (n = t*512 + p*4 + j; 128 large descriptors instead of 512
    small ones).  The halved SBUF write side lets each of the 16 SDMA
    engines run read descriptors back-to-back at its ~27 GB/s cap:
    ~420 GB/s effective per core, finishing all loads in ~177us.
  - preamble: t^T columns as per-(b,c) [128,1] tiles via fused DVE
    scalar_tensor_tensor rows against partition-broadcast text (per-batch
    tiles let PE's batch-0 v chain start after only 4 STTs); v rows / bias
    on the TensorEngine in bf16 (fp32 matmuls are 4x slower and PE boots in
    a low pstate), replicated across partitions with ones-vector matmuls.
  - main loop: one fused DVE scalar_tensor_tensor per 128-row block
    (~1.2us; no DVE fast mode exists for STT, so this is the critical
    engine at ~97% busy).  Per-batch bias is added once on the [128, 32]
    score tile before writeback.
Output is written as [BL, 128, 32] (partition-major) and unshuffled on host.
"""

import os
import sys

import numpy as np

_REPO = "/opt/trn_rl_repo"
if _REPO not in sys.path:
    sys.path.insert(0, _REPO)

B, N, PD, TD, H = 32, 4096, 1024, 768, 512
NCORES = 8
BL = B // NCORES          # batches per core
P = 128                   # partitions
NB = N // P               # 32 n-blocks of 128 rows
JPT = 4                   # n-blocks per DMA tile (2 MiB per DMA)
NT = NB // JPT            # DMA tiles per batch
HC = H // P               # h chunks
PATCH_BUFS = 12

_NC_CACHE = {}
LAST_RESULTS = None       # BassKernelResults of the most recent kernel() call


def _build_nc():
    import concourse.bacc as bacc
    import concourse.bass as bass
    import concourse.mybir as mybir
    from concourse.tile import TileContext

    f32 = mybir.dt.float32
    bf16 = mybir.dt.bfloat16
    mult = mybir.AluOpType.mult
    add = mybir.AluOpType.add

    nc = bacc.Bacc("TRN2", target_bir_lowering=False, debug=False,
                   num_devices=NCORES)

    patches = nc.dram_tensor("patches", [BL, N, PD], f32, kind="ExternalInput")[:]
    text = nc.dram_tensor("text", [BL, TD], f32, kind="ExternalInput")[:]
    w_patch = nc.dram_tensor("w_patch", [H, PD], f32, kind="ExternalInput")[:]
    b_patch = nc.dram_tensor("b_patch", [H], f32, kind="ExternalInput")[:]
    w_text = nc.dram_tensor("w_text", [H, TD], f32, kind="ExternalInput")[:]
    b_text = nc.dram_tensor("b_text", [H], f32, kind="ExternalInput")[:]
    scores = nc.dram_tensor("scores", [BL, P, NB], f32, kind="ExternalOutput")[:]

    with TileContext(nc) as tc:
        with (
            tc.tile_pool(name="const", bufs=1) as const,
            tc.tile_pool(name="patch", bufs=PATCH_BUFS) as ppool,
            tc.tile_pool(name="psum", bufs=1, space=bass.MemorySpace.PSUM) as psum,
        ):
            # ---- small-tensor loads ----
            # Everything rides the gpsimd (SWDGE) ring in priority order,
            # BEFORE any patch-tile DMA is emitted: the ring is FIFO, so the
            # preamble tensors stream at the full engine rate with zero
            # competition instead of being starved behind the patch flood.
            # PE-side operands (W_patch, b_patch, v) are cast to bf16 to
            # dodge the 4x fp32 matmul penalty during the pstate ramp.
            # h = HC*p + c mapping everywhere (tT, W_patch, biases agree, so
            # the contractions are unchanged): each weight matrix loads as
            # ONE DMA whose per-partition span is HC consecutive h rows =
            # 12-16 KiB contiguous.  Big weight packets keep the SDMA
            # engines at their per-descriptor line rate (small 3 KB chunks
            # cost ~4% of stream bandwidth), and 8 preamble emissions
            # instead of 13 start the patch flood ~3us earlier.
            wt_sb = const.tile([P, HC, TD], f32, name="wt")
            nc.gpsimd.dma_start(
                out=wt_sb[:], in_=w_text.rearrange("(p c) td -> p c td", p=P)
            )
            bt_sb = const.tile([P, HC], f32, name="bt_sb")
            nc.gpsimd.dma_start(out=bt_sb[:], in_=b_text.rearrange("(p c) -> p c", p=P))
            # text rows broadcast across partitions (SWDGE replication)
            tx_bc = []
            for b in range(BL):
                t_ = const.tile([P, TD], f32, name=f"txb{b}")
                nc.gpsimd.dma_start(
                    out=t_[:], in_=text[b : b + 1, :].broadcast_to([P, TD])
                )
                tx_bc.append(t_)
            wp_sb = const.tile([P, HC, PD], bf16, name="wp")
            nc.gpsimd.dma_start(
                out=wp_sb[:], in_=w_patch.rearrange("(p c) d -> p c d", p=P)
            )
            bp_sb = const.tile([P, HC], bf16, name="bp_sb")
            nc.gpsimd.dma_start(out=bp_sb[:], in_=b_patch.rearrange("(p c) -> p c", p=P))
            ones128 = const.tile([1, P], bf16, name="ones128")
            nc.vector.memset(ones128[:], 1.0)

            # ---- t^T[h, b] = b_text[h] + sum_td W_text[h, td]*text[b, td] ----
            # Separate [128, 1] tile per (b, c) so the PE v chain for batch 0
            # depends only on batch 0's four STTs (tile-granular tracking).
            tT_sb = [
                [const.tile([P, 1], f32, name=f"tT{b}_{c}") for c in range(HC)]
                for b in range(BL)
            ]
            tT_bf = [
                [const.tile([P, 1], bf16, name=f"tTb{b}_{c}") for c in range(HC)]
                for b in range(BL)
            ]
            prod_t = const.tile([P, TD], f32, name="prod_t")
            for b in range(BL):
                for c in range(HC):
                    nc.vector.scalar_tensor_tensor(
                        out=prod_t[:],
                        in0=wt_sb[:, c, :],
                        scalar=1.0,
                        in1=tx_bc[b][:, :],
                        op0=mult,
                        op1=mult,
                        accum_out=tT_sb[b][c][:, 0:1],
                    )
                    nc.vector.tensor_scalar_add(
                        out=tT_bf[b][c][:, 0:1],
                        in0=tT_sb[b][c][:, 0:1],
                        scalar1=bt_sb[:, c : c + 1],
                    )

            # ---- per-batch v rows + partition broadcast (PE + ACT) ----
            vbc = []
            for b in range(BL):
                v_row = const.tile([1, PD], bf16, name=f"v_row{b}", tag="v_row", bufs=2)
                for half in range(PD // 512):
                    v_ps = psum.tile([1, 512], f32, name=f"v_ps{b}_{half}", tag="v_ps")
                    for c in range(HC):
                        nc.tensor.matmul(
                            v_ps[:],
                            lhsT=tT_bf[b][c][:, 0:1],
                            rhs=wp_sb[:, c, half * 512 : (half + 1) * 512],
                            start=(c == 0),
                            stop=(c == HC - 1),
                        )
                    nc.scalar.copy(
                        out=v_row[0:1, half * 512 : (half + 1) * 512], in_=v_ps[:]
                    )
                vb_sb = const.tile([P, PD], bf16, name=f"vbc{b}")
                for half in range(PD // 512):
                    vb_ps = psum.tile(
                        [P, 512], f32, name=f"vb_ps{b}_{half}", tag="vb_ps", bufs=2
                    )
                    nc.tensor.matmul(
                        vb_ps[:],
                        lhsT=ones128[:],
                        rhs=v_row[0:1, half * 512 : (half + 1) * 512],
                        start=True,
                        stop=True,
                    )
                    nc.scalar.copy(
                        out=vb_sb[:, half * 512 : (half + 1) * 512], in_=vb_ps[:]
                    )
                vbc.append(vb_sb)

            # ---- per-batch bias rows + broadcast ----
            br_sb = const.tile([1, BL], bf16, name="br_sb")
            for b in range(BL):
                br_ps = psum.tile([1, 1], f32, name=f"brp{b}", tag="br_ps")
                for c in range(HC):
                    nc.tensor.matmul(
                        br_ps[:],
                        lhsT=tT_bf[b][c][:, 0:1],
                        rhs=bp_sb[:, c : c + 1],
                        start=(c == 0),
                        stop=(c == HC - 1),
                    )
                nc.scalar.copy(out=br_sb[0:1, b : b + 1], in_=br_ps[:])
            bbc_ps = psum.tile([P, BL], f32, name="bbc_ps", tag="bbc_ps")
            nc.tensor.matmul(
                bbc_ps[:], lhsT=ones128[:], rhs=br_sb[:], start=True, stop=True
            )
            bbc = const.tile([P, BL], f32, name="bbc")
            nc.scalar.copy(out=bbc[:], in_=bbc_ps[:])

            # ---- main loop: fused DVE dot product per 128-row block ----
            # Patches stream via SWDGE cast-DMA (fp32 in HBM -> bf16 in
            # SBUF): the halved SBUF write side lets each SDMA engine run
            # its 16 KiB read descriptors back-to-back at the ~27 GB/s
            # engine cap (~433 GB/s aggregate vs ~380 for plain fp32).
            # Multiply/reduce split across engines.  Measured per block:
            # fused DVE STT (multiply+accum, never packs) = 1.2us; bare DVE
            # tensor_tensor multiply DOES hit the 2x packed-bf16 mode
            # (680ns) with the free-dim reduction on the otherwise-idle ACT
            # engine as a Copy-activation with accum_out (1.16us).  Either
            # path alone leaves one engine critical at ~170us, so blocks
            # alternate: even j -> fused STT on DVE, odd j -> TT(DVE 2x) +
            # ACT reduce.  Per tile that is ~3.8us DVE + ~2.3us ACT, both
            # under the ~4.8us DMA supply cadence -> the kernel is
            # HBM-DMA-bound.  Score writebacks ride the empty sync ring so
            # ACT never stalls behind DVE's per-batch epilogue add.
            prod_stt = const.tile([P, PD], bf16, name="prod_stt")
            for b in range(BL):
                sc_sb = const.tile([P, NB], f32, name=f"sc{b}")
                # n = t*512 + p*4 + j: each partition reads one contiguous
                # 16 KiB span per tile (128 descriptors/tile instead of 512)
                pr = patches[b].rearrange("(t p j) d -> t p j d", p=P, j=JPT)
                for t in range(NT):
                    tile_ = ppool.tile([P, JPT, PD], bf16, tag="ptile", name="ptile")
                    nc.gpsimd.dma_start(out=tile_[:], in_=pr[t])
                    # The very last tile drains pure-STT: the TT->ACT-reduce
                    # handoff would put ACT's 1.16us on the critical exit
                    # path after the final DMA completes.
                    last_tile = b == BL - 1 and t == NT - 1
                    for j in range(JPT):
                        col = t * JPT + j
                        if j % 2 == 0 or last_tile:
                            nc.vector.scalar_tensor_tensor(
                                out=prod_stt[:],
                                in0=tile_[:, j, :],
                                scalar=1.0,
                                in1=vbc[b][:, :],
                                op0=mult,
                                op1=mult,
                                accum_out=sc_sb[:, col : col + 1],
                            )
                        else:
                            prod = const.tile(
                                [P, PD], bf16, name="prod", tag="prod", bufs=2
                            )
                            nc.vector.tensor_tensor(
                                out=prod[:],
                                in0=tile_[:, j, :],
                                in1=vbc[b][:, :],
                                op=mult,
                            )
                            junk = const.tile(
                                [P, PD], bf16, name="ajunk", tag="ajunk", bufs=2
                            )
                            nc.scalar.activation(
                                out=junk[:],
                                in_=prod[:],
                                func=mybir.ActivationFunctionType.Copy,
                                accum_out=sc_sb[:, col : col + 1],
                            )
                nc.vector.tensor_scalar_add(
                    out=sc_sb[:, :], in0=sc_sb[:, :], scalar1=bbc[:, b : b + 1]
                )
                nc.sync.dma_start(out=scores[b], in_=sc_sb[:])

    nc.compile()
    return nc


def _get_nc():
    if "nc" not in _NC_CACHE:
        _NC_CACHE["nc"] = _build_nc()
    return _NC_CACHE["nc"]


def _install_profile_shim():
    """Provide antenv.axon_hooks (NTFF profiling over axon) when absent.

    Replicates trn_agent_boot's ctypes hook against libaxon_pjrt.so so
    run_bass_kernel_spmd(trace=True) can capture device profiles."""
    import contextlib
    import ctypes
    import types

    try:
        from antenv.axon_hooks import get_axon_ntff_profile_hook  # noqa: F401
        return
    except ImportError:
        pass

    so_path = "/opt/axon/libaxon_pjrt.so"
    hook = None
    if os.path.exists(so_path):
        lib = ctypes.CDLL(so_path)
        if hasattr(lib, "axon_start_nrt_profile"):
            lib.axon_start_nrt_profile.argtypes = [
                ctypes.POINTER(ctypes.c_int64),
                ctypes.c_size_t,
            ]
            lib.axon_start_nrt_profile.restype = ctypes.c_int64
            lib.axon_stop_nrt_profile.argtypes = [ctypes.c_char_p]
            lib.axon_stop_nrt_profile.restype = ctypes.c_int64

            @contextlib.contextmanager
            def _hook(output_dir, device_ids):
                import jax

                jax.devices()
                if device_ids:
                    ids = (ctypes.c_int64 * len(device_ids))(*device_ids)
                    rc = lib.axon_start_nrt_profile(ids, len(device_ids))
                else:
                    rc = lib.axon_start_nrt_profile(None, 0)
                if rc != 0:
                    raise RuntimeError(f"axon_start_nrt_profile rc={rc}")
                try:
                    yield
                finally:
                    n = lib.axon_stop_nrt_profile(str(output_dir).encode())
                    print(f"ntff profile: {n} file(s) -> {output_dir}",
                          file=sys.stderr)

            hook = _hook

    mod = types.ModuleType("antenv.axon_hooks")
    mod.get_axon_ntff_profile_hook = lambda: hook
    mod.set_axon_ntff_profile_hook = lambda h: None
    sys.modules["antenv.axon_hooks"] = mod


def kernel(**inputs):
    from concourse.bass_utils import run_bass_kernel_spmd

    global LAST_RESULTS

    patches = np.ascontiguousarray(np.asarray(inputs["patches"], dtype=np.float32))
    text = np.ascontiguousarray(np.asarray(inputs["text"], dtype=np.float32))
    w_patch = np.ascontiguousarray(np.asarray(inputs["W_patch"], dtype=np.float32))
    b_patch = np.ascontiguousarray(np.asarray(inputs["b_patch"], dtype=np.float32))
    w_text = np.ascontiguousarray(np.asarray(inputs["W_text"], dtype=np.float32))
    b_text = np.ascontiguousarray(np.asarray(inputs["b_text"], dtype=np.float32))

    nc = _get_nc()
    in_maps = []
    for c in range(NCORES):
        in_maps.append(
            {
                "patches": patches[c * BL : (c + 1) * BL],
                "text": text[c * BL : (c + 1) * BL],
                "w_patch": w_patch,
                "b_patch": b_patch,
                "w_text": w_text,
                "b_text": b_text,
            }
        )

    trace = bool(int(os.environ.get("KERNEL_PROFILE", "0")))
    if trace:
        _install_profile_shim()
        import concourse.bass_utils as _bu

        _bu.upload_artifacts = lambda tmpdir: ""  # no artifact bucket here
    res = run_bass_kernel_spmd(
        nc, in_maps, core_ids=list(range(NCORES)), trace=trace
    )
    LAST_RESULTS = res

    # scores[b, p, t*JPT + j] holds n = t*(P*JPT) + p*JPT + j
    out = np.concatenate(
        [
            res.results[c]["scores"]
            .reshape(BL, P, NT, JPT)
            .transpose(0, 2, 1, 3)
            .reshape(BL, N)
            for c in range(NCORES)
        ],
        axis=0,
    )
    return out


# revision 52
# speedup vs baseline: 1.1221x; 1.1221x over previous
"""Trainium2 Bass kernel for nn_BilinearScorer.

Reference computation (per full input):
    t = text @ W_text.T + b_text            # [B, H]
    v = t @ W_patch                         # [B, PD]
    scores[b, n] = patches[b, n, :] . v[b]  + t[b] . b_patch   # [B, N]

Strategy: data-parallel over batch B across 8 NeuronCores (4 batches/core).
The heavy op (patches . v) is HBM-bandwidth bound (64 MiB patches per core).
Per core:
  - all loads ride the gpsimd SWDGE ring in priority order (W_text chunk 0,
    b_text, replicated text rows, remaining W_text, W_patch, b_patch, then
    the 32 patch tiles): the ring is FIFO, so preamble tensors stream at
    full rate before the patch flood instead of being starved behind it.
    The ACT ring only writes scores back; ACT itself only does PSUM->SBUF
    copies, so no DMA issue ever blocks on a cross-engine wait.
  - patches are cast fp32->bf16 inline by the SWDGE DMA datapath, and rows
    are remapped so each partition reads one contiguous 16 KiB span per
    2 MiB tile (n = t*512 + p*4 + j; 128 large descriptors instead of 512
    small ones).  The halved SBUF write side lets each of the 16 SDMA
    engines run read descriptors back-to-back at its ~27 GB/s cap:
    ~420 GB/s effective per core, finishing all loads in ~177us.
  - preamble: t^T columns as per-(b,c) [128,1] tiles via fused DVE
    scalar_tensor_tensor rows against partition-broadcast text (per-batch
    tiles let PE's batch-0 v chain start after only 4 STTs); v rows / bias
    on the TensorEngine in bf16 (fp32 matmuls are 4x slower and PE boots in
    a low pstate), replicated across partitions with ones-vector matmuls.
  - main loop: one fused DVE scalar_tensor_tensor per 128-row block
    (~1.2us; no DVE fast mode exists for STT, so this is the critical
    engine at ~97% busy).  Per-batch bias is added once on the [128, 32]
    score tile before writeback.
Output is written as [BL, 128, 32] (partition-major) and unshuffled on host.
"""

import os
import sys

import numpy as np

_REPO = "/opt/trn_rl_repo"
if _REPO not in sys.path:
    sys.path.insert(0, _REPO)

B, N, PD, TD, H = 32, 4096, 1024, 768, 512
NCORES = 8
BL = B // NCORES          # batches per core
P = 128                   # partitions
NB = N // P               # 32 n-blocks of 128 rows
JPT = 4                   # n-blocks per DMA tile (2 MiB per DMA)
NT = NB // JPT            # DMA tiles per batch
HC = H // P               # h chunks
PATCH_BUFS = 12

_NC_CACHE = {}
LAST_RESULTS = None       # BassKernelResults of the most recent kernel() call


def _build_nc():
    import concourse.bacc as bacc
    import concourse.bass as bass
    import concourse.mybir as mybir
    from concourse.tile import TileContext

    f32 = mybir.dt.float32
    bf16 = mybir.dt.bfloat16
    mult = mybir.AluOpType.mult
    add = mybir.AluOpType.add

    nc = bacc.Bacc("TRN2", target_bir_lowering=False, debug=False,
                   num_devices=NCORES)

    patches = nc.dram_tensor("patches", [BL, N, PD], f32, kind="ExternalInput")[:]
    text = nc.dram_tensor("text", [BL, TD], f32, kind="ExternalInput")[:]
    w_patch = nc.dram_tensor("w_patch", [H, PD], f32, kind="ExternalInput")[:]
    b_patch = nc.dram_tensor("b_patch", [H], f32, kind="ExternalInput")[:]
    w_text = nc.dram_tensor("w_text", [H, TD], f32, kind="ExternalInput")[:]
    b_text = nc.dram_tensor("b_text", [H], f32, kind="ExternalInput")[:]
    scores = nc.dram_tensor("scores", [BL, P, NB], f32, kind="ExternalOutput")[:]

    with TileContext(nc) as tc:
        with (
            tc.tile_pool(name="const", bufs=1) as const,
            tc.tile_pool(name="patch", bufs=PATCH_BUFS) as ppool,
            tc.tile_pool(name="psum", bufs=1, space=bass.MemorySpace.PSUM) as psum,
        ):
            # ---- small-tensor loads ----
            # Everything rides the gpsimd (SWDGE) ring in priority order,
            # BEFORE any patch-tile DMA is emitted: the ring is FIFO, so the
            # preamble tensors stream at the full engine rate with zero
            # competition instead of being starved behind the patch flood.
            # PE-side operands (W_patch, b_patch, v) are cast to bf16 to
            # dodge the 4x fp32 matmul penalty during the pstate ramp.
            # h = HC*p + c mapping everywhere (tT, W_patch, biases agree):
            # each weight matrix loads as ONE DMA whose per-partition span
            # is HC consecutive h rows = 12-16 KiB contiguous, keeping the
            # SDMA engines at line rate and starting the patch flood ~3us
            # earlier (8 preamble emissions instead of 13).
            wt_sb = const.tile([P, HC, TD], f32, name="wt")
            nc.gpsimd.dma_start(
                out=wt_sb[:], in_=w_text.rearrange("(p c) td -> p c td", p=P)
            )
            bt_sb = const.tile([P, HC], f32, name="bt_sb")
            nc.gpsimd.dma_start(out=bt_sb[:], in_=b_text.rearrange("(p c) -> p c", p=P))
            # text rows broadcast across partitions (SWDGE replication)
            tx_bc = []
            for b in range(BL):
                t_ = const.tile([P, TD], f32, name=f"txb{b}")
                nc.gpsimd.dma_start(
                    out=t_[:], in_=text[b : b + 1, :].broadcast_to([P, TD])
                )
                tx_bc.append(t_)
            wp_sb = const.tile([P, HC, PD], bf16, name="wp")
            nc.gpsimd.dma_start(
                out=wp_sb[:], in_=w_patch.rearrange("(p c) d -> p c d", p=P)
            )
            bp_sb = const.tile([P, HC], bf16, name="bp_sb")
            nc.gpsimd.dma_start(out=bp_sb[:], in_=b_patch.rearrange("(p c) -> p c", p=P))
            ones128 = const.tile([1, P], bf16, name="ones128")
            nc.vector.memset(ones128[:], 1.0)

            # ---- t^T[h, b] = b_text[h] + sum_td W_text[h, td]*text[b, td] ----
            # Separate [128, 1] tile per (b, c) so the PE v chain for batch 0
            # depends only on batch 0's four STTs (tile-granular tracking).
            tT_sb = [
                [const.tile([P, 1], f32, name=f"tT{b}_{c}") for c in range(HC)]
                for b in range(BL)
            ]
            tT_bf = [
                [const.tile([P, 1], bf16, name=f"tTb{b}_{c}") for c in range(HC)]
                for b in range(BL)
            ]
            prod_t = const.tile([P, TD], f32, name="prod_t")
            for b in range(BL):
                for c in range(HC):
                    nc.vector.scalar_tensor_tensor(
                        out=prod_t[:],
                        in0=wt_sb[:, c, :],
                        scalar=1.0,
                        in1=tx_bc[b][:, :],
                        op0=mult,
                        op1=mult,
                        accum_out=tT_sb[b][c][:, 0:1],
                    )
                    nc.vector.tensor_scalar_add(
                        out=tT_bf[b][c][:, 0:1],
                        in0=tT_sb[b][c][:, 0:1],
                        scalar1=bt_sb[:, c : c + 1],
                    )

            # ---- per-batch v rows + partition broadcast (PE + ACT) ----
            vbc = []
            for b in range(BL):
                v_row = const.tile([1, PD], bf16, name=f"v_row{b}", tag="v_row", bufs=2)
                for half in range(PD // 512):
                    v_ps = psum.tile([1, 512], f32, name=f"v_ps{b}_{half}", tag="v_ps")
                    for c in range(HC):
                        nc.tensor.matmul(
                            v_ps[:],
                            lhsT=tT_bf[b][c][:, 0:1],
                            rhs=wp_sb[:, c, half * 512 : (half + 1) * 512],
                            start=(c == 0),
                            stop=(c == HC - 1),
                        )
                    nc.scalar.copy(
                        out=v_row[0:1, half * 512 : (half + 1) * 512], in_=v_ps[:]
                    )
                vb_sb = const.tile([P, PD], bf16, name=f"vbc{b}")
                for half in range(PD // 512):
                    vb_ps = psum.tile(
                        [P, 512], f32, name=f"vb_ps{b}_{half}", tag="vb_ps", bufs=2
                    )
                    nc.tensor.matmul(
                        vb_ps[:],
                        lhsT=ones128[:],
                        rhs=v_row[0:1, half * 512 : (half + 1) * 512],
                        start=True,
                        stop=True,
                    )
                    nc.scalar.copy(
                        out=vb_sb[:, half * 512 : (half + 1) * 512], in_=vb_ps[:]
                    )
                vbc.append(vb_sb)

            # ---- per-batch bias rows + broadcast ----
            br_sb = const.tile([1, BL], bf16, name="br_sb")
            for b in range(BL):
                br_ps = psum.tile([1, 1], f32, name=f"brp{b}", tag="br_ps")
                for c in range(HC):
                    nc.tensor.matmul(
                        br_ps[:],
                        lhsT=tT_bf[b][c][:, 0:1],
                        rhs=bp_sb[:, c : c + 1],
                        start=(c == 0),
                        stop=(c == HC - 1),
                    )
                nc.scalar.copy(out=br_sb[0:1, b : b + 1], in_=br_ps[:])
            bbc_ps = psum.tile([P, BL], f32, name="bbc_ps", tag="bbc_ps")
            nc.tensor.matmul(
                bbc_ps[:], lhsT=ones128[:], rhs=br_sb[:], start=True, stop=True
            )
            bbc = const.tile([P, BL], f32, name="bbc")
            nc.scalar.copy(out=bbc[:], in_=bbc_ps[:])

            # ---- main loop: fused DVE dot product per 128-row block ----
            # Patches stream via SWDGE cast-DMA (fp32 in HBM -> bf16 in
            # SBUF): the halved SBUF write side lets each SDMA engine run
            # its 16 KiB read descriptors back-to-back at the ~27 GB/s
            # engine cap (~433 GB/s aggregate vs ~380 for plain fp32).
            # Multiply/reduce split across engines.  Measured per block:
            # fused DVE STT (multiply+accum, never packs) = 1.2us; bare DVE
            # tensor_tensor multiply DOES hit the 2x packed-bf16 mode
            # (680ns) with the free-dim reduction on the otherwise-idle ACT
            # engine as a Copy-activation with accum_out (1.16us).  Either
            # path alone leaves one engine critical at ~170us, so blocks
            # alternate: even j -> fused STT on DVE, odd j -> TT(DVE 2x) +
            # ACT reduce.  Per tile that is ~3.8us DVE + ~2.3us ACT, both
            # under the ~4.8us DMA supply cadence -> the kernel is
            # HBM-DMA-bound.  Score writebacks ride the empty sync ring so
            # ACT never stalls behind DVE's per-batch epilogue add.
            prod_stt = const.tile([P, PD], bf16, name="prod_stt")
            for b in range(BL):
                sc_sb = const.tile([P, NB], f32, name=f"sc{b}")
                # n = t*512 + p*4 + j: each partition reads one contiguous
                # 16 KiB span per tile (128 descriptors/tile instead of 512)
                pr = patches[b].rearrange("(t p j) d -> t p j d", p=P, j=JPT)
                for t in range(NT):
                    tile_ = ppool.tile([P, JPT, PD], bf16, tag="ptile", name="ptile")
                    nc.gpsimd.dma_start(out=tile_[:], in_=pr[t])
                    # Last tile drains pure-STT (no ACT handoff on the exit path)
                    last_tile = b == BL - 1 and t == NT - 1
                    for j in range(JPT):
                        col = t * JPT + j
                        if j % 2 == 0 or last_tile:
                            nc.vector.scalar_tensor_tensor(
                                out=prod_stt[:],
                                in0=tile_[:, j, :],
                                scalar=1.0,
                                in1=vbc[b][:, :],
                                op0=mult,
                                op1=mult,
                                accum_out=sc_sb[:, col : col + 1],
                            )
                        else:
                            prod = const.tile(
                                [P, PD], bf16, name="prod", tag="prod", bufs=2
                            )
                            nc.vector.tensor_tensor(
                                out=prod[:],
                                in0=tile_[:, j, :],
                                in1=vbc[b][:, :],
                                op=mult,
                            )
                            junk = const.tile(
                                [P, PD], bf16, name="ajunk", tag="ajunk", bufs=2
                            )
                            nc.scalar.activation(
                                out=junk[:],
                                in_=prod[:],
                                func=mybir.ActivationFunctionType.Copy,
                                accum_out=sc_sb[:, col : col + 1],
                            )
                nc.vector.tensor_scalar_add(
                    out=sc_sb[:, :], in0=sc_sb[:, :], scalar1=bbc[:, b : b + 1]
                )
                nc.sync.dma_start(out=scores[b], in_=sc_sb[:])

    nc.compile()
    return nc


def _get_nc():
    if "nc" not in _NC_CACHE:
        _NC_CACHE["nc"] = _build_nc()
    return _NC_CACHE["nc"]


def _install_profile_shim():
    """Provide antenv.axon_hooks (NTFF profiling over axon) when absent.

    Replicates trn_agent_boot's ctypes hook against libaxon_pjrt.so so
    run_bass_kernel_spmd(trace=True) can capture device profiles."""
    import contextlib
    import ctypes
    import types

    try:
        from antenv.axon_hooks import get_axon_ntff_profile_hook  # noqa: F401
        return
    except ImportError:
        pass

    so_path = "/opt/axon/libaxon_pjrt.so"
    hook = None
    if os.path.exists(so_path):
        lib = ctypes.CDLL(so_path)
        if hasattr(lib, "axon_start_nrt_profile"):
            lib.axon_start_nrt_profile.argtypes = [
                ctypes.POINTER(ctypes.c_int64),
                ctypes.c_size_t,
            ]
            lib.axon_start_nrt_profile.restype = ctypes.c_int64
            lib.axon_stop_nrt_profile.argtypes = [ctypes.c_char_p]
            lib.axon_stop_nrt_profile.restype = ctypes.c_int64

            @contextlib.contextmanager
            def _hook(output_dir, device_ids):
                import jax

                jax.devices()
                if device_ids:
                    ids = (ctypes.c_int64 * len(device_ids))(*device_ids)
                    rc = lib.axon_start_nrt_profile(ids, len(device_ids))
                else:
                    rc = lib.axon_start_nrt_profile(None, 0)
                if rc != 0:
                    raise RuntimeError(f"axon_start_nrt_profile rc={rc}")
                try:
                    yield
                finally:
                    n = lib.axon_stop_nrt_profile(str(output_dir).encode())
                    print(f"ntff profile: {n} file(s) -> {output_dir}",
                          file=sys.stderr)

            hook = _hook

    mod = types.ModuleType("antenv.axon_hooks")
    mod.get_axon_ntff_profile_hook = lambda: hook
    mod.set_axon_ntff_profile_hook = lambda h: None
    sys.modules["antenv.axon_hooks"] = mod


def kernel(**inputs):
    from concourse.bass_utils import run_bass_kernel_spmd

    global LAST_RESULTS

    patches = np.ascontiguousarray(np.asarray(inputs["patches"], dtype=np.float32))
    text = np.ascontiguousarray(np.asarray(inputs["text"], dtype=np.float32))
    w_patch = np.ascontiguousarray(np.asarray(inputs["W_patch"], dtype=np.float32))
    b_patch = np.ascontiguousarray(np.asarray(inputs["b_patch"], dtype=np.float32))
    w_text = np.ascontiguousarray(np.asarray(inputs["W_text"], dtype=np.float32))
    b_text = np.ascontiguousarray(np.asarray(inputs["b_text"], dtype=np.float32))

    nc = _get_nc()
    in_maps = []
    for c in range(NCORES):
        in_maps.append(
            {
                "patches": patches[c * BL : (c + 1) * BL],
                "text": text[c * BL : (c + 1) * BL],
                "w_patch": w_patch,
                "b_patch": b_patch,
                "w_text": w_text,
                "b_text": b_text,
            }
        )

    trace = bool(int(os.environ.get("KERNEL_PROFILE", "0")))
    if trace:
        _install_profile_shim()
        import concourse.bass_utils as _bu

        _bu.upload_artifacts = lambda tmpdir: ""  # no artifact bucket here
    res = run_bass_kernel_spmd(
        nc, in_maps, core_ids=list(range(NCORES)), trace=trace
    )
    LAST_RESULTS = res

    # scores[b, p, t*JPT + j] holds n = t*(P*JPT) + p*JPT + j
    out = np.concatenate(
        [
            res.results[c]["scores"]
            .reshape(BL, P, NT, JPT)
            .transpose(0, 2, 1, 3)
            .reshape(BL, N)
            for c in range(NCORES)
        ],
        axis=0,
    )
    return out
